# revision 9
# baseline (speedup 1.0000x reference)
"""Corr1d-x-group cost-volume kernel for Trainium2 (8 NeuronCores, SPMD).

Data-parallel over batch N=8: core i processes batch i.

Per core (inputs [16, 256, 512] f32 each, output [108, 256, 512] f32):
  out[g*27+ch, h, w] = 0.25 * sum_c f1[g*4+c, h, w] * f2[g*4+c, h, w+ch-23]
with zero padding outside w in [0, 512).

v4 design (v3 trace: DVE products at 2291ns/4096-elem as predicted, 4-way
col-tiled matmuls overlap (864 MMs -> 113us wall), but 216 fine-grained
stores serialized 2.5us each on the Sync HWDGE queue = 91% busy):
  - SBUF partition layout per group g: (c4, grp32) = 128 where partition
    (c, grp) holds channel g*4+c, rows [grp*8, grp*8+8) -- each partition
    covers 8 CONSECUTIVE rows, so a group loads as one [128 x 4096]
    contiguous-per-partition cast DMA and the fp16 tensor_tensor product
    for a (group, ch) pair is ONE instruction with free dim 4096.
  - f2 dual parity copies keep every shifted slice 4B-aligned (DVE 2x_1P
    mode); the odd-parity copy is built on-chip by a ScalarE copy.
  - Channels processed in batches of 4: the TensorE reduction col-tiles
    the 4 channels of a batch into the 4 32-partition strips of a PSUM
    bank (partition = 32*cc + grp, m = grp, block-diagonal 0.25 weight).
    Per (g, batch): 4 products, 32 matmuls (8 row-slices j x 4 concurrent
    strips), 4 ScalarE 2-bank evacuations into a [128, 4096] staging tile
    whose flat order is (cc, grp, hl, w) = the DRAM element order.
  - One 2MB store per (g, batch): DRAM side is [4 channels x 512KB
    contiguous] -> 2-dim access pattern, ~0.5us HWDGE issue (28 stores
    total vs 216).
"""

import os
import numpy as np

import concourse.bass as bass
import concourse.bacc as bacc
import concourse.mybir as mybir
import concourse.tile as tile
from concourse import bass_utils

N, C, H, W = 8, 16, 256, 512
G = 4
TOP_CH = 27
OUT_CH = G * TOP_CH  # 108
GRP = 32  # 8-row groups per H (partition dim factor)
HB = 8  # rows per group (free dim factor)
SEG = 544  # padded f2 segment width per row (even -> 4B-aligned starts)
PADL = 24  # f2 even tile: column = w + PADL
FD = HB * W  # 4096 free elements per product / staging tile

_CACHED = {}


def _reduction_weights() -> np.ndarray:
    # lhsT [K=(c4, grp32)=128, M=grp'=32]: sums the 4 channels of the group
    # at row-group grp and applies the 1/sumelems scale.
    w = np.zeros((128, 32), np.float16)
    for c in range(G):
        for grp in range(GRP):
            w[c * GRP + grp, grp] = 0.25
    return w


def _build_program() -> bass.Bass:
    # Bacc (not raw Bass): its compile() splits multi-sem sync waits, which
    # TRN2 hardware limits to one per instruction.
    nc = bacc.Bacc(
        "TRN2",
        target_bir_lowering=False,
        debug=False,
        enable_asserts=False,
        num_devices=N,
    )
    f16 = mybir.dt.float16
    f32 = mybir.dt.float32

    l_in = nc.dram_tensor("l_in", [C, H, W], f32, kind="ExternalInput")
    r_in = nc.dram_tensor("r_in", [C, H, W], f32, kind="ExternalInput")
    w_red = nc.dram_tensor("w_red", [128, 32], f16, kind="ExternalInput")
    out = nc.dram_tensor("out", [OUT_CH, H, W], f32, kind="ExternalOutput")

    # Input viewed as [g, (c grp)=128, (hb w)=4096]: partition (c, grp)
    # holds 8 consecutive rows -> contiguous 16KB DRAM chunks per partition.
    l_v = l_in.ap().rearrange(
        "(g c) (grp hb) w -> g (c grp) (hb w)", g=G, c=G, grp=GRP, hb=HB
    )
    r_v = r_in.ap().rearrange(
        "(g c) (grp hb) w -> g (c grp) (hb w)", g=G, c=G, grp=GRP, hb=HB
    )

    # Output viewed as [g, c, (grp hl w)]: per (g, 4-channel batch) the
    # store destination is [4 channels x 512KB contiguous] = a 2-dim AP.
    out_v = out.ap().rearrange(
        "(g c) (grp hl) w -> g c (grp hl w)", g=G, grp=GRP, hl=HB
    )

    # Channel batches of 4 (last batch has 3).
    batches = [list(range(b, min(b + 4, TOP_CH))) for b in range(0, TOP_CH, 4)]

    with tile.TileContext(nc) as tc:
        with (
            tc.tile_pool(name="wpool", bufs=1) as wpool,
            tc.tile_pool(name="inpool", bufs=2) as inpool,
            tc.tile_pool(name="prodpool", bufs=6) as prodpool,
            tc.tile_pool(name="obpool", bufs=3) as obpool,
            tc.tile_pool(name="psumpool", bufs=4, space="PSUM") as psumpool,
        ):
            wt = wpool.tile([128, 32], f16)
            nc.sync.dma_start(wt[:], w_red[:])

            for g in range(G):
                f1 = inpool.tile([128, FD], f16, tag="f1")
                nc.gpsimd.dma_start(f1[:], l_v[g : g + 1])

                f2e = inpool.tile([128, HB, SEG], f16, tag="f2e")
                nc.vector.memset(f2e[:, :, 0:PADL], 0.0)
                nc.vector.memset(f2e[:, :, PADL + W : SEG], 0.0)
                nc.gpsimd.dma_start(f2e[:, :, PADL : PADL + W], r_v[g : g + 1])
                # Odd-parity copy: same data at column w + (PADL-1). Loaded
                # via its own cast DMA (DMA bandwidth has slack; an on-chip
                # ScalarE copy queues behind the PSUM evacuations on the
                # near-saturated ACT FIFO and stalls DVE at group switches).
                f2o = inpool.tile([128, HB, SEG], f16, tag="f2o")
                nc.vector.memset(f2o[:, :, 0 : PADL - 1], 0.0)
                nc.vector.memset(f2o[:, :, PADL - 1 + W : SEG], 0.0)
                nc.gpsimd.dma_start(
                    f2o[:, :, PADL - 1 : PADL - 1 + W], r_v[g : g + 1]
                )

                for chans in batches:
                    nch = len(chans)
                    prods = []
                    for ch in chans:
                        # shift s2 = ch - 23; slice start col in the parity
                        # tile is even -> 4B-aligned -> DVE 2x_1P fp16 mode.
                        if ch % 2 == 1:
                            src = f2e[:, :, ch + 1 : ch + 1 + W]
                        else:
                            src = f2o[:, :, ch : ch + W]
                        p = prodpool.tile([128, FD], f16, tag="prod")
                        nc.vector.tensor_mul(p[:], f1[:], src)
                        prods.append(p)

                    # 8 row-slices j -> 4 two-bank PSUM tiles (pairs of j);
                    # channel cc of the batch goes to partition strip cc.
                    pss = [
                        psumpool.tile([128, 2 * W], f32, tag="ps", name=f"ps{jp}")
                        for jp in range(4)
                    ]
                    for j in range(HB):
                        jp, jl = divmod(j, 2)
                        for cc in range(nch):
                            nc.tensor.matmul(
                                pss[jp][
                                    32 * cc : 32 * (cc + 1),
                                    jl * W : (jl + 1) * W,
                                ],
                                wt[:],
                                prods[cc][:, j * W : (j + 1) * W],
                                start=True,
                                stop=True,
                                tile_position=(0, 32 * cc),
                            )
                    # Staging tile: partition (cc, grp), free (hl=j, w) --
                    # flat order (cc, grp, hl, w) matches DRAM layout.
                    ob = obpool.tile([128, HB, W], f32, tag="ob")
                    for jp in range(4):
                        nc.scalar.copy(
                            ob[:, 2 * jp : 2 * jp + 2, :], pss[jp][:]
                        )
                    nc.sync.dma_start(
                        out_v[g : g + 1, chans[0] : chans[0] + nch],
                        ob[: 32 * nch],
                    )
    nc.compile()
    return nc


def kernel(l_in: np.ndarray, r_in: np.ndarray) -> np.ndarray:
    assert l_in.shape == (N, C, H, W) and r_in.shape == (N, C, H, W)
    l_in = np.ascontiguousarray(l_in, dtype=np.float32)
    r_in = np.ascontiguousarray(r_in, dtype=np.float32)

    if "nc" not in _CACHED:
        _CACHED["nc"] = _build_program()
    nc = _CACHED["nc"]

    w_np = _reduction_weights()
    in_maps = [
        {
            "l_in": np.ascontiguousarray(l_in[i]),
            "r_in": np.ascontiguousarray(r_in[i]),
            "w_red": w_np,
        }
        for i in range(N)
    ]
    trace = bool(int(os.environ.get("CORR_KERNEL_TRACE", "0")))
    kwargs = {}
    tdir = os.environ.get("CORR_KERNEL_TRACE_DIR")
    if trace and tdir:
        os.makedirs(tdir, exist_ok=True)
        kwargs["tmpdir"] = tdir
    res = bass_utils.run_bass_kernel_spmd(
        nc, in_maps, core_ids=list(range(N)), trace=trace, **kwargs
    )
    _CACHED["last_result"] = res
    return np.stack([res.results[i]["out"] for i in range(N)], axis=0)


# revision 10
# speedup vs baseline: 1.1123x; 1.1123x over previous
"""Corr1d-x-group cost-volume kernel for Trainium2 (8 NeuronCores, SPMD).

Data-parallel over batch N=8: core i processes batch i.

Per core (inputs [16, 256, 512] f32 each, output [108, 256, 512] f32):
  out[g*27+ch, h, w] = 0.25 * sum_c f1[g*4+c, h, w] * f2[g*4+c, h, w+ch-23]
with zero padding outside w in [0, 512).

v4 design (v3 trace: DVE products at 2291ns/4096-elem as predicted, 4-way
col-tiled matmuls overlap (864 MMs -> 113us wall), but 216 fine-grained
stores serialized 2.5us each on the Sync HWDGE queue = 91% busy):
  - SBUF partition layout per group g: (c4, grp32) = 128 where partition
    (c, grp) holds channel g*4+c, rows [grp*8, grp*8+8) -- each partition
    covers 8 CONSECUTIVE rows, so a group loads as one [128 x 4096]
    contiguous-per-partition cast DMA and the fp16 tensor_tensor product
    for a (group, ch) pair is ONE instruction with free dim 4096.
  - f2 dual parity copies keep every shifted slice 4B-aligned (DVE 2x_1P
    mode); the odd-parity copy is built on-chip by a ScalarE copy.
  - Channels processed in batches of 4: the TensorE reduction col-tiles
    the 4 channels of a batch into the 4 32-partition strips of a PSUM
    bank (partition = 32*cc + grp, m = grp, block-diagonal 0.25 weight).
    Per (g, batch): 4 products, 32 matmuls (8 row-slices j x 4 concurrent
    strips), 4 ScalarE 2-bank evacuations into a [128, 4096] staging tile
    whose flat order is (cc, grp, hl, w) = the DRAM element order.
  - One 2MB store per (g, batch): DRAM side is [4 channels x 512KB
    contiguous] -> 2-dim access pattern, ~0.5us HWDGE issue (28 stores
    total vs 216).
"""

import os
import numpy as np

import concourse.bass as bass
import concourse.bacc as bacc
import concourse.mybir as mybir
import concourse.tile as tile
from concourse import bass_utils

N, C, H, W = 8, 16, 256, 512
G = 4
TOP_CH = 27
OUT_CH = G * TOP_CH  # 108
GRP = 32  # 8-row groups per H (partition dim factor)
HB = 8  # rows per group (free dim factor)
SEG = 544  # padded f2 segment width per row (even -> 4B-aligned starts)
PADL = 24  # f2 even tile: column = w + PADL
FD = HB * W  # 4096 free elements per product / staging tile

_CACHED = {}


def _reduction_weights() -> np.ndarray:
    # lhsT [K=(c4, grp32)=128, M=grp'=32]: sums the 4 channels of the group
    # at row-group grp and applies the 1/sumelems scale.
    w = np.zeros((128, 32), np.float16)
    for c in range(G):
        for grp in range(GRP):
            w[c * GRP + grp, grp] = 0.25
    return w


def _build_program() -> bass.Bass:
    # Bacc (not raw Bass): its compile() splits multi-sem sync waits, which
    # TRN2 hardware limits to one per instruction.
    nc = bacc.Bacc(
        "TRN2",
        target_bir_lowering=False,
        debug=False,
        enable_asserts=False,
        num_devices=N,
    )
    f16 = mybir.dt.float16
    f32 = mybir.dt.float32

    l_in = nc.dram_tensor("l_in", [C, H, W], f32, kind="ExternalInput")
    r_in = nc.dram_tensor("r_in", [C, H, W], f32, kind="ExternalInput")
    w_red = nc.dram_tensor("w_red", [128, 32], f16, kind="ExternalInput")
    out = nc.dram_tensor("out", [OUT_CH, H, W], f32, kind="ExternalOutput")

    # Input viewed as [g, (c grp)=128, (hb w)=4096]: partition (c, grp)
    # holds 8 consecutive rows -> contiguous 16KB DRAM chunks per partition.
    l_v = l_in.ap().rearrange(
        "(g c) (grp hb) w -> g (c grp) (hb w)", g=G, c=G, grp=GRP, hb=HB
    )
    r_v = r_in.ap().rearrange(
        "(g c) (grp hb) w -> g (c grp) (hb w)", g=G, c=G, grp=GRP, hb=HB
    )

    # Output viewed as [g, c, (grp hl w)]: per (g, 4-channel batch) the
    # store destination is [4 channels x 512KB contiguous] = a 2-dim AP.
    out_v = out.ap().rearrange(
        "(g c) (grp hl) w -> g c (grp hl w)", g=G, grp=GRP, hl=HB
    )

    # Channel batches of 4 (last batch has 3).
    batches = [list(range(b, min(b + 4, TOP_CH))) for b in range(0, TOP_CH, 4)]

    with tile.TileContext(nc) as tc:
        with (
            tc.tile_pool(name="wpool", bufs=1) as wpool,
            tc.tile_pool(name="inpool", bufs=2) as inpool,
            tc.tile_pool(name="prodpool", bufs=6) as prodpool,
            tc.tile_pool(name="obpool", bufs=3) as obpool,
            tc.tile_pool(name="psumpool", bufs=4, space="PSUM") as psumpool,
        ):
            wt = wpool.tile([128, 32], f16)
            nc.sync.dma_start(wt[:], w_red[:])

            for g in range(G):
                f1 = inpool.tile([128, FD], f16, tag="f1")
                nc.gpsimd.dma_start(f1[:], l_v[g : g + 1])

                f2e = inpool.tile([128, HB, SEG], f16, tag="f2e")
                nc.vector.memset(f2e[:, :, 0:PADL], 0.0)
                nc.vector.memset(f2e[:, :, PADL + W : SEG], 0.0)
                nc.gpsimd.dma_start(f2e[:, :, PADL : PADL + W], r_v[g : g + 1])
                # Odd-parity copy: same data at column w + (PADL-1), built
                # on-chip by a DVE 4x-mode fp16 copy (~1.1us). A second HBM
                # cast-DMA slowed every engine ~15-20% (power throttle from
                # the extra traffic); a ScalarE copy stalled DVE at group
                # switches behind the PSUM evacuations in the ACT FIFO.
                f2o = inpool.tile([128, HB, SEG], f16, tag="f2o")
                nc.vector.memset(f2o[:, :, 0 : PADL - 1], 0.0)
                nc.vector.memset(f2o[:, :, PADL - 1 + W : SEG], 0.0)
                nc.vector.tensor_copy(
                    f2o[:, :, PADL - 1 : PADL - 1 + W],
                    f2e[:, :, PADL : PADL + W],
                )

                for chans in batches:
                    nch = len(chans)
                    prods = []
                    for ch in chans:
                        # shift s2 = ch - 23; slice start col in the parity
                        # tile is even -> 4B-aligned -> DVE 2x_1P fp16 mode.
                        if ch % 2 == 1:
                            src = f2e[:, :, ch + 1 : ch + 1 + W]
                        else:
                            src = f2o[:, :, ch : ch + W]
                        p = prodpool.tile([128, FD], f16, tag="prod")
                        nc.vector.tensor_mul(p[:], f1[:], src)
                        prods.append(p)

                    # 8 row-slices j -> 4 two-bank PSUM tiles (pairs of j);
                    # channel cc of the batch goes to partition strip cc.
                    pss = [
                        psumpool.tile([128, 2 * W], f32, tag="ps", name=f"ps{jp}")
                        for jp in range(4)
                    ]
                    for j in range(HB):
                        jp, jl = divmod(j, 2)
                        for cc in range(nch):
                            nc.tensor.matmul(
                                pss[jp][
                                    32 * cc : 32 * (cc + 1),
                                    jl * W : (jl + 1) * W,
                                ],
                                wt[:],
                                prods[cc][:, j * W : (j + 1) * W],
                                start=True,
                                stop=True,
                                tile_position=(0, 32 * cc),
                            )
                    # Staging tile: partition (cc, grp), free (hl=j, w) --
                    # flat order (cc, grp, hl, w) matches DRAM layout.
                    ob = obpool.tile([128, HB, W], f32, tag="ob")
                    for jp in range(4):
                        nc.scalar.copy(
                            ob[:, 2 * jp : 2 * jp + 2, :], pss[jp][:]
                        )
                    nc.sync.dma_start(
                        out_v[g : g + 1, chans[0] : chans[0] + nch],
                        ob[: 32 * nch],
                    )
    nc.compile()
    return nc


def kernel(l_in: np.ndarray, r_in: np.ndarray) -> np.ndarray:
    assert l_in.shape == (N, C, H, W) and r_in.shape == (N, C, H, W)
    l_in = np.ascontiguousarray(l_in, dtype=np.float32)
    r_in = np.ascontiguousarray(r_in, dtype=np.float32)

    if "nc" not in _CACHED:
        _CACHED["nc"] = _build_program()
    nc = _CACHED["nc"]

    w_np = _reduction_weights()
    in_maps = [
        {
            "l_in": np.ascontiguousarray(l_in[i]),
            "r_in": np.ascontiguousarray(r_in[i]),
            "w_red": w_np,
        }
        for i in range(N)
    ]
    trace = bool(int(os.environ.get("CORR_KERNEL_TRACE", "0")))
    kwargs = {}
    tdir = os.environ.get("CORR_KERNEL_TRACE_DIR")
    if trace and tdir:
        os.makedirs(tdir, exist_ok=True)
        kwargs["tmpdir"] = tdir
    res = bass_utils.run_bass_kernel_spmd(
        nc, in_maps, core_ids=list(range(N)), trace=trace, **kwargs
    )
    _CACHED["last_result"] = res
    return np.stack([res.results[i]["out"] for i in range(N)], axis=0)
